# revision 7
# baseline (speedup 1.0000x reference)
"""Trainium2 Bass kernel for nn_MetaGAT: 5-layer edge-featured GAT (2 heads).

Strategy (8 NeuronCores, dst-node sharding):
- Pad N=10000 -> 10240 nodes = 80 blocks of 128; core c owns blocks [10c, 10c+10)
  (node rows [1280c, 1280c+1280)).
- Host: append self-loops, sort edges by dst, bucket into the dst-owning
  128-node block, pad each block to a fixed capacity (data-independent IR).
- Per layer on device:
  Phase A (replicated): xw = x @ W and per-node attention scalars
  s_i/s_j written to a DRAM table [10240, 320] (cols 0:256 xw, 256:258 s_j,
  258:260 s_i).
  Phase B (sharded by dst): per block, dma_gather the per-edge rows by src,
  expand s_i via per-chunk one-hot matmuls, compute w = exp(leakyrelu(alpha))
  without segment-max (alpha is bounded; softmax is shift-invariant), fold w
  into per-chunk one-hot "S'" matrices and segment-reduce messages via
  TensorE matmuls accumulating in PSUM: per-head features, edge-attr
  aggregates (factorized: sum(w*attr) @ ew^T replaces per-edge edge_emb), and
  denominators. Normalize, combine heads, bias (+ReLU).
- Layers bridged with an 8-core AllGather of the [1280,128] shards.
"""

import sys

sys.path.insert(0, "/opt/trn_rl_repo")

import numpy as np

import concourse.bacc as bacc
import concourse.bass as bass
import concourse.mybir as mybir
import concourse.tile as tile
from concourse import bass_utils
from concourse.masks import make_identity

F32 = mybir.dt.float32
I16 = mybir.dt.int16
I32 = mybir.dt.int32
AF = mybir.ActivationFunctionType
OP = mybir.AluOpType

P = 128
D = 128
H = 2
F = 16
NEG_SLOPE = 0.2
EPS = 1e-16


class Cfg:
    def __init__(self, n_nodes=10000, n_layers=5, cores=8, blocks_per_core=10,
                 chunks_per_block=37):
        self.N = n_nodes
        self.L = n_layers
        self.CORES = cores
        self.BPC = blocks_per_core
        self.NBLK = cores * blocks_per_core
        self.NPAD = self.NBLK * P
        assert self.NPAD >= self.N
        self.CH = chunks_per_block
        self.CAP = chunks_per_block * P      # edge slots per block
        self.TBL = 320                        # table cols (elem bytes % 256 == 0)
        self.ROWS_PER_CORE = self.BPC * P


FULL_CFG = Cfg()


# ----------------------------------------------------------------------------
# Device program
# ----------------------------------------------------------------------------

def build_program(cfg: Cfg) -> bacc.Bacc:
    nc = bacc.Bacc("TRN2", target_bir_lowering=False, debug=False,
                   num_devices=cfg.CORES)
    L, BPC, CH, CAP, NPAD, TBL = cfg.L, cfg.BPC, cfg.CH, cfg.CAP, cfg.NPAD, cfg.TBL

    # ---- external tensors (identical shapes on every core) ----
    x0 = nc.dram_tensor("x0", [NPAD, D], F32, kind="ExternalInput")
    Wt = nc.dram_tensor("Wt", [L, D, H * D], F32, kind="ExternalInput")
    Bt = nc.dram_tensor("Bt", [L, D, 4], F32, kind="ExternalInput")
    ewT = nc.dram_tensor("ewT", [L, H, F + 1, D], F32, kind="ExternalInput")
    brep = nc.dram_tensor("brep", [L, P, D], F32, kind="ExternalInput")
    attr_t = nc.dram_tensor("attr_t", [BPC, P, CH * (F + 1)], F32, kind="ExternalInput")
    es_t = nc.dram_tensor("es_t", [L, BPC, P, CH * H], F32, kind="ExternalInput")
    idx_w = nc.dram_tensor("idx_w", [BPC, P, CAP // 16], I16, kind="ExternalInput")
    nid_w = nc.dram_tensor("nid_w", [BPC, P, P // 16], I16, kind="ExternalInput")
    dl_col = nc.dram_tensor("dl_col", [BPC, P, CH], F32, kind="ExternalInput")
    dl_row = nc.dram_tensor("dl_row", [BPC, CAP], F32, kind="ExternalInput")
    out_ext = nc.dram_tensor("out", [cfg.ROWS_PER_CORE, D], F32, kind="ExternalOutput")

    with tile.TileContext(nc) as tc:
        with (
            tc.tile_pool(name="const", bufs=1) as cpool,
            tc.tile_pool(name="resident", bufs=1) as rpool,
            tc.tile_pool(name="dram", bufs=1, space="DRAM") as dpool,
            tc.tile_pool(name="agdram", bufs=2, space="DRAM") as agpool,
            tc.tile_pool(name="gath", bufs=2) as gpool,
        ):
            # ---- constants ----
            iota_i = cpool.tile([P, P], I32)
            nc.gpsimd.iota(iota_i[:], pattern=[[1, P]], base=0, channel_multiplier=0)
            iota_row = cpool.tile([P, P], F32)
            nc.vector.tensor_copy(iota_row[:], iota_i[:])
            niota_i = cpool.tile([P, 1], I32)
            nc.gpsimd.iota(niota_i[:], pattern=[[1, 1]], base=0, channel_multiplier=-1)
            neg_iota = cpool.tile([P, 1], F32)
            nc.vector.tensor_copy(neg_iota[:], niota_i[:])
            ones_col = cpool.tile([P, 1], F32)
            nc.vector.memset(ones_col[:], 1.0)
            ones_row = cpool.tile([1, P], F32)
            nc.vector.memset(ones_row[:], 1.0)
            ident = cpool.tile([P, P], F32)
            make_identity(nc, ident[:])
            # zeros used to clear PSUM banks via a start=True matmul
            zro = cpool.tile([P, 2 * (D + 1)], F32)
            nc.vector.memset(zro[:], 0.0)

            # ---- resident inputs ----
            W_sb = rpool.tile([P, L * H * D], F32)
            for l in range(L):
                nc.sync.dma_start(W_sb[:, l * H * D:(l + 1) * H * D], Wt[l])
            B_sb = rpool.tile([P, L * 4], F32)
            for l in range(L):
                nc.sync.dma_start(B_sb[:, l * 4:(l + 1) * 4], Bt[l])
            ewT_sb = rpool.tile([F + 1, L * H * D], F32)
            for l in range(L):
                for h in range(H):
                    nc.sync.dma_start(
                        ewT_sb[:, (l * H + h) * D:(l * H + h + 1) * D], ewT[l, h])
            brep_sb = rpool.tile([P, L * D], F32)
            for l in range(L):
                nc.sync.dma_start(brep_sb[:, l * D:(l + 1) * D], brep[l])
            attr_sb = rpool.tile([P, BPC * CH * (F + 1)], F32)
            for bb in range(BPC):
                nc.sync.dma_start(
                    attr_sb[:, bb * CH * (F + 1):(bb + 1) * CH * (F + 1)], attr_t[bb])
            dlc_sb = rpool.tile([P, BPC * CH], F32)
            for bb in range(BPC):
                nc.sync.dma_start(dlc_sb[:, bb * CH:(bb + 1) * CH], dl_col[bb])

            # ---- DRAM scratch ----
            table = dpool.tile([NPAD, TBL], F32)
            # zero the pad columns once (phase A only writes cols 0:260)
            padz = cpool.tile([P, TBL - (H * D + 4)], F32)
            nc.vector.memset(padz[:], 0.0)
            for t in range(cfg.NBLK):
                nc.sync.dma_start(
                    table[t * P:(t + 1) * P, H * D + 4:TBL], padz[:])

            xsrc = x0.ap()
            for l in range(L):
                last = l == L - 1
                # ============ Phase A: build table ============
                with (
                    tc.tile_pool(name="pa", bufs=3) as pa,
                    tc.tile_pool(name="pa_ps", bufs=2, space="PSUM") as pa_ps,
                ):
                    for t in range(cfg.NBLK):
                        x_sb = pa.tile([P, D], F32, tag="x_sb")
                        nc.sync.dma_start(x_sb[:], xsrc[t * P:(t + 1) * P, :])
                        xT_ps = pa_ps.tile([P, P], F32, tag="xT_ps")
                        nc.tensor.transpose(xT_ps[:], x_sb[:], ident[:])
                        xT_sb = pa.tile([P, P], F32, tag="xT_sb")
                        nc.vector.tensor_copy(xT_sb[:], xT_ps[:])
                        xw_ps = pa_ps.tile([P, H * D + 4], F32, tag="xw_ps")
                        nc.tensor.matmul(
                            xw_ps[:, 0:H * D], lhsT=xT_sb[:],
                            rhs=W_sb[:, l * H * D:(l + 1) * H * D],
                            start=True, stop=True)
                        nc.tensor.matmul(
                            xw_ps[:, H * D:H * D + 4], lhsT=xT_sb[:],
                            rhs=B_sb[:, l * 4:(l + 1) * 4],
                            start=True, stop=True)
                        asm_sb = pa.tile([P, H * D + 4], F32, tag="asm_sb")
                        nc.vector.tensor_copy(asm_sb[:], xw_ps[:])
                        nc.sync.dma_start(
                            table[t * P:(t + 1) * P, 0:H * D + 4], asm_sb[:])

                # ============ Phase B: edge aggregation ============
                if not last:
                    ag_in = agpool.tile([cfg.ROWS_PER_CORE, D], F32, tag="ag_in")
                    ag_out = agpool.tile(
                        [NPAD, D], F32, tag="ag_out",
                        addr_space="Shared" if cfg.CORES > 4 else "Local")
                with (
                    tc.tile_pool(name="pb", bufs=2) as pb,
                    tc.tile_pool(name="pbs", bufs=3) as pbs,
                    tc.tile_pool(name="pb_feat", bufs=2, space="PSUM") as ps_feat,
                    tc.tile_pool(name="pb_attr", bufs=2, space="PSUM") as ps_attr,
                    tc.tile_pool(name="pb_si", bufs=2, space="PSUM") as ps_si,
                    tc.tile_pool(name="pb_rep", bufs=2, space="PSUM") as ps_rep,
                ):
                    for bb in range(BPC):
                        idx_sb = pb.tile([P, CAP // 16], I16, tag="idx_sb")
                        nc.sync.dma_start(idx_sb[:], idx_w[bb])
                        nid_sb = pb.tile([P, P // 16], I16, tag="nid_sb")
                        nc.sync.dma_start(nid_sb[:], nid_w[bb])
                        dlr_sb = pb.tile([1, CAP], F32, tag="dlr_sb")
                        nc.sync.dma_start(dlr_sb[:], dl_row[bb:bb + 1, :])
                        es_sb = pb.tile([P, CH * H], F32, tag="es_sb")
                        nc.sync.dma_start(es_sb[:], es_t[l, bb])

                        gath = gpool.tile([P, CH, TBL], F32, tag="gath")
                        nc.gpsimd.dma_gather(
                            gath[:], table[:], idx_sb[:],
                            num_idxs=CAP, num_idxs_reg=CAP, elem_size=TBL,
                            single_packet=False)
                        si_g = pb.tile([P, 1, TBL], F32, tag="si_g")
                        nc.gpsimd.dma_gather(
                            si_g[:], table[:], nid_sb[:],
                            num_idxs=P, num_idxs_reg=P, elem_size=TBL)

                        # ---- s_i expansion: one-hot^T matmuls per chunk ----
                        siexp_ps = ps_si.tile([P, H * CH], F32, tag="siexp")
                        for c in range(CH):
                            rep_ps = ps_rep.tile([P, P], F32, tag="rep")
                            nc.tensor.matmul(
                                rep_ps[:], lhsT=ones_row[:],
                                rhs=dlr_sb[0:1, c * P:(c + 1) * P],
                                start=True, stop=True)
                            S_T = pbs.tile([P, P], F32, tag="S_T")
                            nc.scalar.activation(S_T[:], rep_ps[:], AF.Abs,
                                                 bias=neg_iota[:], scale=1.0)
                            nc.scalar.activation(S_T[:], S_T[:], AF.Relu,
                                                 bias=1.0, scale=-1.0)
                            nc.tensor.matmul(
                                siexp_ps[:, H * c:H * (c + 1)], lhsT=S_T[:],
                                rhs=si_g[:, 0, H * D + 2:H * D + 4],
                                start=True, stop=True)

                        # ---- alpha -> w ----
                        alpha_sb = pb.tile([P, CH * H], F32, tag="alpha_sb")
                        nc.vector.tensor_tensor(
                            alpha_sb[:], siexp_ps[:],
                            gath[:, :, H * D:H * D + 2], op=OP.add)
                        nc.vector.tensor_tensor(
                            alpha_sb[:], alpha_sb[:], es_sb[:], op=OP.add)
                        # leaky_relu(x) = max(0.2*x, x); then exp
                        w_sb = pb.tile([P, CH * H], F32, tag="w_sb")
                        nc.vector.scalar_tensor_tensor(
                            w_sb[:], in0=alpha_sb[:], scalar=NEG_SLOPE,
                            in1=alpha_sb[:], op0=OP.mult, op1=OP.max)
                        nc.scalar.activation(w_sb[:], w_sb[:], AF.Exp)

                        # ---- segment reduction matmuls ----
                        # One start=True matmul clears the whole PSUM bank
                        # (has_written semantics); all real matmuls then
                        # accumulate with start=False (first touch per
                        # element overwrites).
                        feat_ps = ps_feat.tile([P, 2 * (D + 1)], F32, tag="feat")
                        attr_ps = ps_attr.tile([F + 1, H * D], F32, tag="attr")
                        nc.tensor.matmul(
                            feat_ps[:], lhsT=zro[:, 0:P], rhs=zro[:],
                            start=True, stop=False)
                        nc.tensor.matmul(
                            attr_ps[:], lhsT=zro[:, 0:F + 1], rhs=zro[:, 0:H * D],
                            start=True, stop=False)
                        for c in range(CH):
                            abase = (bb * CH + c) * (F + 1)
                            for h in range(H):
                                Sp = pbs.tile([P, P], F32, tag=f"Sp{h}")
                                nc.vector.scalar_tensor_tensor(
                                    Sp[:], in0=iota_row[:],
                                    scalar=dlc_sb[:, bb * CH + c:bb * CH + c + 1],
                                    in1=w_sb[:, H * c + h:H * c + h + 1].to_broadcast([P, P]),
                                    op0=OP.is_equal, op1=OP.mult)
                                fb = h * (D + 1)
                                nc.tensor.matmul(
                                    feat_ps[:, fb:fb + D], lhsT=Sp[:],
                                    rhs=gath[:, c, h * D:(h + 1) * D],
                                    start=False, stop=False)
                                nc.tensor.matmul(
                                    feat_ps[:, fb + D:fb + D + 1], lhsT=Sp[:],
                                    rhs=ones_col[:],
                                    start=False, stop=False)
                                nc.tensor.matmul(
                                    attr_ps[:, h * D:h * D + P],
                                    lhsT=attr_sb[:, abase:abase + F + 1],
                                    rhs=Sp[:],
                                    start=False,
                                    stop=(c == CH - 1 and h == H - 1))

                        # ---- epilogue: attr-factorized term, normalize ----
                        attr_agg = pb.tile([F + 1, H * D], F32, tag="attr_agg")
                        nc.vector.tensor_copy(attr_agg[:], attr_ps[:])
                        for h in range(H):
                            nc.tensor.matmul(
                                feat_ps[:, h * (D + 1):h * (D + 1) + D],
                                lhsT=attr_agg[:, h * D:h * D + P],
                                rhs=ewT_sb[:, (l * H + h) * D:(l * H + h + 1) * D],
                                start=False, stop=(h == H - 1))
                        denom_sb = pb.tile([P, H], F32, tag="denom_sb")
                        # 2*(denom+eps): fold the head-mean 0.5 into the reciprocal
                        nc.vector.tensor_scalar(
                            denom_sb[:], in0=feat_ps[:, D:2 * D + 2:D + 1],
                            scalar1=2.0, scalar2=2.0 * EPS, op0=OP.mult, op1=OP.add)
                        rec_sb = pb.tile([P, H], F32, tag="rec_sb")
                        nc.vector.reciprocal(rec_sb[:], denom_sb[:])
                        t0 = pb.tile([P, D], F32, tag="t0")
                        nc.vector.tensor_scalar(
                            t0[:], in0=feat_ps[:, 0:D], scalar1=rec_sb[:, 0:1],
                            scalar2=None, op0=OP.mult)
                        t1 = pb.tile([P, D], F32, tag="t1")
                        nc.vector.scalar_tensor_tensor(
                            t1[:], in0=feat_ps[:, D + 1:2 * D + 1],
                            scalar=rec_sb[:, 1:2], in1=t0[:],
                            op0=OP.mult, op1=OP.add)
                        out_sb = pb.tile([P, D], F32, tag="out_sb")
                        nc.vector.tensor_tensor(
                            out_sb[:], t1[:], brep_sb[:, l * D:(l + 1) * D],
                            op=OP.add)
                        if not last:
                            nc.vector.tensor_scalar(
                                out_sb[:], in0=out_sb[:], scalar1=0.0,
                                scalar2=None, op0=OP.max)
                            nc.sync.dma_start(
                                ag_in[bb * P:(bb + 1) * P, :], out_sb[:])
                        else:
                            nc.sync.dma_start(
                                out_ext[bb * P:(bb + 1) * P, :], out_sb[:])

                if not last:
                    nc.gpsimd.collective_compute(
                        "AllGather", OP.bypass,
                        replica_groups=[list(range(cfg.CORES))],
                        ins=[ag_in.opt()], outs=[ag_out.opt()])
                    xsrc = ag_out[:]

    nc.compile()
    return nc


# ----------------------------------------------------------------------------
# Host preparation
# ----------------------------------------------------------------------------

def host_prep(cfg: Cfg, x, edge_index, edge_attr, weights, atts, biases,
              edge_ws, edge_bs):
    N, L, CORES, BPC, CH, CAP, NPAD = (cfg.N, cfg.L, cfg.CORES, cfg.BPC,
                                       cfg.CH, cfg.CAP, cfg.NPAD)
    NBLK = cfg.NBLK
    x = np.asarray(x, np.float32)
    ei = np.asarray(edge_index).astype(np.int64)
    ea = np.asarray(edge_attr, np.float32)
    Ws = np.asarray(weights, np.float32)
    atts = np.asarray(atts, np.float32)
    bs = np.asarray(biases, np.float32)
    ews = np.asarray(edge_ws, np.float32)
    ebs = np.asarray(edge_bs, np.float32)

    E = ei.shape[1]
    src = np.concatenate([ei[0], np.arange(N, dtype=np.int64)])
    dst = np.concatenate([ei[1], np.arange(N, dtype=np.int64)])
    eaf = np.concatenate([ea, np.zeros((N, F), np.float32)])

    # es[l, e, h] = edge_attr[e] @ (ew_lh^T @ att_j_lh) + eb_lh @ att_j_lh
    es_all = np.zeros((L, E + N, H), np.float32)
    for l in range(L):
        att_j = atts[l][:, D:]                        # [H, D]
        ew3 = ews[l].reshape(H, D, F)
        eb2 = ebs[l].reshape(H, D)
        for h in range(H):
            qe = ew3[h].T @ att_j[h]                  # [F]
            ceb = eb2[h] @ att_j[h]
            es_all[l, :, h] = eaf @ qe + ceb

    order = np.argsort(dst, kind="stable")
    dst_s, src_s, eaf_s = dst[order], src[order], eaf[order]
    es_s = es_all[:, order]

    # block bucketing (fixed capacity)
    blk_of = dst_s // P
    starts = np.searchsorted(blk_of, np.arange(NBLK + 1))
    counts = starts[1:] - starts[:-1]
    if counts.max() > CAP:
        raise ValueError(f"block capacity {CAP} exceeded: max {counts.max()}")

    srcidx = np.zeros((NBLK, CAP), np.int64)
    dstloc = np.full((NBLK, CAP), float(P), np.float32)
    attr17 = np.zeros((NBLK, CAP, F + 1), np.float32)
    es_blk = np.zeros((L, NBLK, CAP, H), np.float32)
    for b in range(NBLK):
        s, e = starts[b], starts[b + 1]
        n = e - s
        srcidx[b, :n] = src_s[s:e]
        dstloc[b, :n] = (dst_s[s:e] - b * P).astype(np.float32)
        attr17[b, :n, :F] = eaf_s[s:e]
        attr17[b, :n, F] = 1.0
        es_blk[:, b, :n, :] = es_s[:, s:e, :]

    def wrap16(idx2d):  # [B, K] int -> [B, 128, K//16] int16 wrapped+replicated
        B, K = idx2d.shape
        w = idx2d.reshape(B, K // 16, 16).transpose(0, 2, 1)  # [B, 16, K//16]
        return np.tile(w, (1, 8, 1)).astype(np.int16)

    idx_w = wrap16(srcidx)                                         # [NBLK,128,CAP/16]
    nids = (np.arange(NBLK * P).reshape(NBLK, P)).astype(np.int64)
    nid_w = wrap16(nids)                                           # [NBLK,128,8]
    dl_col = dstloc.reshape(NBLK, CH, P).transpose(0, 2, 1).copy() # [NBLK,128,CH]
    dl_row = dstloc                                                # [NBLK, CAP]
    attr_t = attr17.reshape(NBLK, CH, P, F + 1).transpose(0, 2, 1, 3) \
                   .reshape(NBLK, P, CH * (F + 1)).copy()
    es_t = es_blk.reshape(L, NBLK, CH, P, H).transpose(0, 1, 3, 2, 4) \
                 .reshape(L, NBLK, P, CH * H).copy()

    # weights: Wt natural; Bt = W @ A with cols [j0, j1, i0, i1]
    Bt = np.zeros((L, D, 4), np.float32)
    for l in range(L):
        att_i = atts[l][:, :D]
        att_j = atts[l][:, D:]
        A = np.zeros((H * D, 4), np.float32)
        A[0:D, 0] = att_j[0]
        A[D:2 * D, 1] = att_j[1]
        A[0:D, 2] = att_i[0]
        A[D:2 * D, 3] = att_i[1]
        Bt[l] = Ws[l] @ A
    ewT17 = np.zeros((L, H, F + 1, D), np.float32)
    for l in range(L):
        ew3 = ews[l].reshape(H, D, F)
        eb2 = ebs[l].reshape(H, D)
        for h in range(H):
            ewT17[l, h, :F] = ew3[h].T
            ewT17[l, h, F] = eb2[h]
    brep = np.broadcast_to(bs[:, None, :], (L, P, D)).copy()
    x0 = np.zeros((NPAD, D), np.float32)
    x0[:N] = x

    shared = {
        "x0": x0, "Wt": Ws, "Bt": Bt, "ewT": ewT17, "brep": brep,
    }
    in_maps = []
    for c in range(CORES):
        bsl = slice(c * BPC, (c + 1) * BPC)
        in_maps.append({
            **shared,
            "attr_t": attr_t[bsl].copy(),
            "es_t": es_t[:, bsl].copy(),
            "idx_w": idx_w[bsl].copy(),
            "nid_w": nid_w[bsl].copy(),
            "dl_col": dl_col[bsl].copy(),
            "dl_row": dl_row[bsl].copy(),
        })
    return in_maps


_PROGRAM_CACHE: dict = {}


def get_program(cfg: Cfg) -> bacc.Bacc:
    key = (cfg.NPAD, cfg.L, cfg.CORES, cfg.BPC, cfg.CH)
    if key not in _PROGRAM_CACHE:
        _PROGRAM_CACHE[key] = build_program(cfg)
    return _PROGRAM_CACHE[key]


def run(cfg: Cfg, inputs: dict, **runkw):
    in_maps = host_prep(cfg, **inputs)
    nc = get_program(cfg)
    res = bass_utils.run_bass_kernel_spmd(
        nc, in_maps, core_ids=list(range(cfg.CORES)), **runkw)
    parts = [res.results[c]["out"] for c in range(cfg.CORES)]
    full = np.concatenate(parts, axis=0)[:cfg.N]
    return full, res


def kernel(**inputs) -> np.ndarray:
    out, _ = run(FULL_CFG, inputs)
    return out.astype(np.float32)
